# revision 21
# baseline (speedup 1.0000x reference)
"""Multi-head attention (B=2, S=2048, E=1024, H=16, Dh=64) on 8 TRN2 NeuronCores.

Sharding: batch x head-group data/tensor parallel. Core c handles batch c//4
and heads [4*(c%4), 4*(c%4)+4): it computes Q/K/V projections for its 256
feature columns, full attention for its 4 heads, and a partial output
projection against its 256 rows of W_o. The host sums the 4 partials per
batch (the "all-reduce after W_o" step of the sharding hint, done at
unshard time) and concatenates the two batches.

Numerics: the whole pre-softmax path runs in float32r (~2^-12 per-element
input rounding, fp32 accumulate). The resulting score error is ~0.3 in
scaled-score units; a noise study against the reference shows that level
of score noise costs ~8e-3 output Frobenius error (gate is 2e-2). The row
max m comes from a q-major f32r score pass reduced on DVE with fused
tensor_tensor_reduce pairs; it only needs to land within ~80 raw units of
the true max (any common shift cancels in softmax normalization). The
k-major score matmul subtracts m via an augmented contraction row
(kT row 64 = 1, qT row 64 = -m), so exp() fuses the PSUM->SBUF copy on
ScalarE with scale=1/sqrt(Dh). The softmax denominator comes free from an
appended ones-column on V; normalization is applied after the P@V matmul.
P is fp16 (post-softmax weights), V/att/W_o are f32r.
"""

from contextlib import ExitStack

import numpy as np

import concourse.bacc as bacc
import concourse.mybir as mybir
import concourse.tile as tile
from concourse import bass_utils
from concourse.masks import make_identity

AF = mybir.ActivationFunctionType
ALU = mybir.AluOpType
F32 = mybir.dt.float32
F16 = mybir.dt.float16
F32R = mybir.dt.float32r

B, S, E, H, Dh = 2, 2048, 1024, 16, 64
NCORES = 8
GROUPS = 4            # head groups (cores per batch)
HPC = H // GROUPS     # heads per core = 4
FG = HPC * Dh         # feature columns per core = 256
P = 128
SCALE = 1.0 / (Dh ** 0.5)

EO = E // P           # 8 contraction chunks
ST = S // P           # 16 sequence tiles of 128
QC = 512              # q-chunk width for the k-major score/PV pass
NQC = S // QC         # 4
NEG_INF = -3.0e38


def _emit(tc, debug=False):
    nc = tc.nc
    xt = nc.dram_tensor("xt", [E, S], F32R, kind="ExternalInput").ap()
    wq = nc.dram_tensor("wq", [E, FG], F32R, kind="ExternalInput").ap()
    wk = nc.dram_tensor("wk", [E, FG], F32R, kind="ExternalInput").ap()
    wv = nc.dram_tensor("wv", [E, FG], F32R, kind="ExternalInput").ap()
    wo = nc.dram_tensor("wo", [FG, E], F32R, kind="ExternalInput").ap()
    out = nc.dram_tensor("out", [S, E], F32, kind="ExternalOutput").ap()

    ctx = ExitStack()
    const = ctx.enter_context(tc.tile_pool(name="const", bufs=1))
    persist = ctx.enter_context(tc.tile_pool(name="persist", bufs=1))
    stage = ctx.enter_context(tc.tile_pool(name="stage", bufs=3))
    xqp = ctx.enter_context(tc.tile_pool(name="xqp", bufs=2))
    stgp = ctx.enter_context(tc.tile_pool(name="stgp", bufs=3))
    ptp = ctx.enter_context(tc.tile_pool(name="ptp", bufs=4))
    outp = ctx.enter_context(tc.tile_pool(name="outp", bufs=4))
    ps_stat = ctx.enter_context(tc.tile_pool(name="ps_stat", bufs=1, space="PSUM"))
    ps_sc = ctx.enter_context(tc.tile_pool(name="ps_sc", bufs=2, space="PSUM"))
    ps_pv = ctx.enter_context(tc.tile_pool(name="ps_pv", bufs=2, space="PSUM"))
    ps_wo = ctx.enter_context(tc.tile_pool(name="ps_wo", bufs=2, space="PSUM"))

    ident = const.tile([P, P], F32)
    make_identity(nc, ident[:])
    ones_f32 = const.tile([P, Dh], F32)
    nc.gpsimd.memset(ones_f32[:], 1.0)
    ones_mat = const.tile([P, Dh], F32R)
    nc.vector.tensor_copy(ones_mat[:], ones_f32[:])

    # persistent SBUF tensors
    wqs = persist.tile([P, EO, FG], F32R)
    wks = persist.tile([P, EO, FG], F32R)
    wvs = persist.tile([P, EO, FG], F32R)
    wos = persist.tile([P, FG // P, E], F32R)
    # per-head Q^T/K^T (partitions 0-63 data; row 64 = -m on Q, ones on K)
    qT = persist.tile([P, HPC, S], F32R)
    kT = persist.tile([P, HPC, S], F32R)
    # V with appended ones column, k-major (partitions = sequence position)
    vau = persist.tile([P, ST, HPC, Dh + 1], F16)
    # normalized attention output, feature-major: feature fc*128+p, q free
    att = persist.tile([P, FG // P, S], F32R)

    xt_re = xt.rearrange("(eo p) s -> p eo s", p=P)
    _wks_re = wk.rearrange("(eo p) m -> p eo m", p=P)
    nc.sync.dma_start(wks[:, 0:2, :], _wks_re[:, 0:2, :])
    nc.sync.dma_start(wks[:, 2:, :], _wks_re[:, 2:, :])

    # K-aug row = 1 so the q-side aug row (-m, raw units) lands in every score
    nc.gpsimd.memset(kT[Dh : Dh + 1, :, :].bitcast(F32), 1.0)
    nc.gpsimd.memset(vau[:, :, :, Dh : Dh + 1], 1.0)

    maxs = [
        stage.tile([P, ST], F32, tag=f"maxs{h}", name=f"maxs{h}") for h in range(HPC)
    ]

    # q-major f32r score pass for the row max: 1024-wide DVE max-reduces
    def stats_steps(h, qt_order=None):
        mx = maxs[h]
        for qt in qt_order or range(ST):
            hm = stage.tile([P, 2], F32, tag="hm")
            for pair in range(2):
                ps2 = ps_stat.tile([P, 2, 512], F32, tag="stat", name="ps_stat")
                for sub in range(2):
                    kc = pair * 2 + sub
                    yield lambda ps2=ps2, sub=sub, qt=qt, h=h, kc=kc: nc.tensor.matmul(
                        ps2[:, sub, :],
                        lhsT=qT[0:Dh, h, qt * P : (qt + 1) * P],
                        rhs=kT[0:Dh, h, kc * 512 : (kc + 1) * 512],
                        start=True,
                        stop=True,
                    )
                nc.vector.reduce_max(
                    hm[:, pair : pair + 1], ps2[:], axis=mybir.AxisListType.XY
                )
            nc.vector.tensor_reduce(
                mx[:, qt : qt + 1], hm[:, 0:2], axis=mybir.AxisListType.X,
                op=ALU.max,
            )

    def drain(it, n=1 << 30):
        k = 0
        if it is not None:
            for step in it:
                step()
                k += 1
                if k >= n:
                    break

    Q_CHUNK_ORDER = [2, 3, 1, 0]
    qt0_order = [qc4 * 4 + j for qc4 in Q_CHUNK_ORDER for j in range(4)]
    stats_its = [stats_steps(0, qt0_order)] + [stats_steps(h) for h in range(1, HPC)]

    # transpose the per-head row-max vector into the qT aug row (negated)
    def aug_prep(h):
        psm = ps_wo.tile([P, QC], F32, tag="wo", name="psm")
        nc.tensor.transpose(psm[0:ST, 0:P], maxs[h][:, :], ident[:])
        mst = stage.tile([ST, P], F32R, tag="mst")
        nc.scalar.mul(mst[:], psm[0:ST, 0:P], -1.0)
        nc.sync.dma_start(qT[Dh : Dh + 1, h, :], mst[:, :])

    # ---- phase 1: K + V projections share each x chunk
    xq_tiles = {}
    for qc4 in range(NQC):
        xq = xqp.tile([P, EO, QC], F32R, tag="xq")
        xq_tiles[qc4] = xq
        qs = slice(qc4 * QC, (qc4 + 1) * QC)
        if qc4 == 0:
            for e2 in range(0, EO, 2):
                nc.sync.dma_start(xq[:, e2 : e2 + 2, :], xt_re[:, e2 : e2 + 2, qs])
            # late weights ride the HWDGE queue behind the first x chunk;
            # they must be EMITTED before the first V/Q matmuls that read them
            nc.gpsimd.dma_start(wvs[:], wv.rearrange("(eo p) m -> p eo m", p=P))
            nc.gpsimd.dma_start(wqs[:], wq.rearrange("(eo p) m -> p eo m", p=P))
            nc.gpsimd.dma_start(wos[:], wo.rearrange("(fo p) e -> p fo e", p=P))
        else:
            nc.sync.dma_start(xq[:], xt_re[:, :, qs])
        for mc in range(FG // P):
            ps = ps_sc.tile([P, QC], F32, tag="sc", name="ps_kproj")
            for eo in range(EO):
                nc.tensor.matmul(
                    ps,
                    lhsT=wks[:, eo, mc * P : (mc + 1) * P],
                    rhs=xq[:, eo, :],
                    start=(eo == 0),
                    stop=(eo == EO - 1),
                )
            stg = stgp.tile([P, QC], F32R, tag="stg")
            nc.vector.tensor_copy(stg[:], ps)
            for hh in range(2):
                h = mc * 2 + hh
                nc.sync.dma_start(kT[0:Dh, h, qs], stg[hh * Dh : (hh + 1) * Dh, :])
        for st4 in range(4):
            st = qc4 * 4 + st4
            ps = ps_pv.tile([P, QC], F32, tag="pv", name="ps_v")[:, :FG]
            for eo in range(EO):
                nc.tensor.matmul(
                    ps,
                    lhsT=xq[:, eo, st4 * P : (st4 + 1) * P],
                    rhs=wvs[:, eo, :],
                    start=(eo == 0),
                    stop=(eo == EO - 1),
                )
            nc.scalar.copy(
                vau[:, st, :, 0:Dh],
                ps.rearrange("p (h d) -> p h d", h=HPC),
            )

    # ---- phase 2: Q projection; chunks 2,3 still resident in the xq ring.
    # head-0 stats drain here (16 per chunk, matching qt availability).
    for ci, qc4 in enumerate(Q_CHUNK_ORDER):
        if qc4 in (2, 3):
            xq = xq_tiles[qc4]
        else:
            xq = xqp.tile([P, EO, QC], F32R, tag="xq")
            nc.sync.dma_start(xq[:], xt_re[:, :, qc4 * QC : (qc4 + 1) * QC])
        qs = slice(qc4 * QC, (qc4 + 1) * QC)
        for mc in range(FG // P):
            ps = ps_sc.tile([P, QC], F32, tag="sc", name="ps_qproj")
            for eo in range(EO):
                nc.tensor.matmul(
                    ps,
                    lhsT=wqs[:, eo, mc * P : (mc + 1) * P],
                    rhs=xq[:, eo, :],
                    start=(eo == 0),
                    stop=(eo == EO - 1),
                )
            stg = stgp.tile([P, QC], F32R, tag="stg")
            nc.vector.tensor_copy(stg[:], ps)
            for hh in range(2):
                h = mc * 2 + hh
                nc.sync.dma_start(qT[0:Dh, h, qs], stg[hh * Dh : (hh + 1) * Dh, :])
            drain(stats_its[0], 8)
    drain(stats_its[0])
    aug_prep(0)

    # ---- per head: k-major scores -> exp -> PV, software-pipelined so PE
    # never sits behind the Act exp; heads 2/3 stats and the W_o projection
    # interleave into the PE slack of the Act-bound chain.
    def wo_steps(qt_lo, qt_hi):
        for qt in range(qt_lo, qt_hi):
            for ec in range(E // QC):
                ps = ps_wo.tile([P, QC], F32, tag="wo", name="ps_wo")
                for fc in range(FG // P):
                    yield lambda ps=ps, qt=qt, ec=ec, fc=fc: nc.tensor.matmul(
                        ps,
                        lhsT=att[:, fc, qt * P : (qt + 1) * P],
                        rhs=wos[:, fc, ec * QC : (ec + 1) * QC],
                        start=(fc == 0),
                        stop=(fc == FG // P - 1),
                        skip_group_check=True,
                    )
                ob = outp.tile([P, QC], F32, tag="ob")
                nc.vector.tensor_copy(ob[:], ps)
                nc.sync.dma_start(
                    out[qt * P : (qt + 1) * P, ec * QC : (ec + 1) * QC], ob[:]
                )

    LAG = 2
    head_order = [0, 1, 2, 3]
    # Fill schedule: head h's section drains head h+1's stats, one step per
    # kt slot (64 steps / 64 slots); the last head drains W_o instead.
    fills = {0: stats_its[1], 1: stats_its[2], 2: stats_its[3]}
    wo_pending = None
    for hi_idx, h in enumerate(head_order):
        for qc in range(NQC):
            qs = slice(qc * QC, (qc + 1) * QC)
            pv = ps_pv.tile([P, QC], F32, tag="pv")
            pts = {}

            def pv_mm(kt, pv=pv, h=h):
                nc.tensor.matmul(
                    pv[0 : Dh + 1, :],
                    lhsT=vau[:, kt, h, :],
                    rhs=pts.pop(kt)[:],
                    start=(kt == 0),
                    stop=(kt == ST - 1),
                    skip_group_check=True,
                )

            for kt in range(ST):
                ks = slice(kt * P, (kt + 1) * P)
                if hi_idx == HPC - 1:
                    drain(wo_pending, 1)
                else:
                    drain(fills[h], 1)
                ps = ps_sc.tile([P, QC], F32, tag="sc")
                nc.tensor.matmul(
                    ps, lhsT=kT[0 : Dh + 1, h, ks], rhs=qT[0 : Dh + 1, h, qs],
                    start=True, stop=True,
                )
                pt = ptp.tile([P, QC], F16, tag="pt")
                pts[kt] = pt
                nc.scalar.activation(pt[:], ps[:], AF.Exp, scale=SCALE)
                if kt >= LAG:
                    pv_mm(kt - LAG)
            for kt in range(ST - LAG, ST):
                pv_mm(kt)
            li = stage.tile([P, QC], F32R, tag="li")
            with nc.allow_low_precision(reason="1/l in f32r (~2^-12) is ample"):
                nc.vector.reciprocal(li[Dh : Dh + 1, :], pv[Dh : Dh + 1, :])
            pb = ps_wo.tile([P, QC], F32, tag="wo", name="pb")
            nc.tensor.matmul(
                pb[0:Dh, :], lhsT=ones_mat[Dh : Dh + 1, :], rhs=li[Dh : Dh + 1, :],
                start=True, stop=True,
            )
            bc = stage.tile([P, QC], F32, tag="bc")
            nc.scalar.copy(bc[0:Dh, :], pb[0:Dh, :])
            if h % 2 == 0:
                nc.vector.tensor_tensor(
                    att[0:Dh, h // 2, qs], pv[0:Dh, :], bc[0:Dh, :], ALU.mult
                )
            else:
                stg = stage.tile([P, QC], F32R, tag="att_stg")
                nc.vector.tensor_tensor(stg[0:Dh, :], pv[0:Dh, :], bc[0:Dh, :], ALU.mult)
                nc.sync.dma_start(att[Dh : 2 * Dh, h // 2, qs], stg[0:Dh, :])
            if hi_idx == HPC - 1:
                drain(wo_pending)
                wo_pending = wo_steps(4 * qc, 4 * qc + 4)
        if hi_idx == HPC - 1:
            drain(wo_pending)
        if hi_idx + 1 < HPC:
            nxt = head_order[hi_idx + 1]
            drain(stats_its[nxt])
            aug_prep(nxt)

    if debug:
        vau_d = nc.dram_tensor("vau_d", [P, ST, HPC, Dh + 1], F16, kind="ExternalOutput").ap()
        nc.sync.dma_start(vau_d, vau[:])
        att_d = nc.dram_tensor("att_d", [P, FG // P, S], F32, kind="ExternalOutput").ap()
        qT_d = nc.dram_tensor("qT_d", [P, HPC, S], F32, kind="ExternalOutput").ap()
        kT_d = nc.dram_tensor("kT_d", [P, HPC, S], F32, kind="ExternalOutput").ap()
        nc.sync.dma_start(att_d, att[:].bitcast(F32))
        nc.sync.dma_start(qT_d, qT[:].bitcast(F32))
        nc.sync.dma_start(kT_d, kT[:].bitcast(F32))
    ctx.close()


_NC = None


def _build(debug=False):
    global _NC
    if debug:
        nc = bacc.Bacc(
            "TRN2", target_bir_lowering=False, debug=False, num_devices=NCORES
        )
        with tile.TileContext(nc) as tc:
            _emit(tc, debug=True)
        nc.compile()
        return nc
    if _NC is None:
        nc = bacc.Bacc(
            "TRN2", target_bir_lowering=False, debug=False, num_devices=NCORES
        )
        with tile.TileContext(nc) as tc:
            _emit(tc)
        nc.compile()
        _NC = nc
    return _NC


def _prep_inputs(x, W_q, W_k, W_v, W_o):
    x = np.asarray(x, dtype=np.float32)
    W_q = np.asarray(W_q, dtype=np.float32)
    W_k = np.asarray(W_k, dtype=np.float32)
    W_v = np.asarray(W_v, dtype=np.float32)
    W_o = np.asarray(W_o, dtype=np.float32)

    xts = [np.ascontiguousarray(x[b].T) for b in range(B)]
    in_maps = []
    for c in range(NCORES):
        b, g = divmod(c, GROUPS)
        fg = slice(g * FG, (g + 1) * FG)
        in_maps.append(
            {
                "xt": xts[b],
                "wq": np.ascontiguousarray(W_q[:, fg]),
                "wk": np.ascontiguousarray(W_k[:, fg]),
                "wv": np.ascontiguousarray(W_v[:, fg]),
                "wo": np.ascontiguousarray(W_o[fg, :]),
            }
        )
    return in_maps


def run(inputs, **spmd_kwargs):
    nc = _build()
    in_maps = _prep_inputs(
        inputs["x"], inputs["W_q"], inputs["W_k"], inputs["W_v"], inputs["W_o"]
    )
    res = bass_utils.run_bass_kernel_spmd(
        nc, in_maps, core_ids=list(range(NCORES)), **spmd_kwargs
    )
    out = np.zeros((B, S, E), dtype=np.float32)
    for c in range(NCORES):
        out[c // GROUPS] += res.results[c]["out"]
    return out, res


def kernel(**inputs):
    out, _ = run(inputs)
    return out


# revision 22
# speedup vs baseline: 1.1615x; 1.1615x over previous
"""Multi-head attention (B=2, S=2048, E=1024, H=16, Dh=64) on 8 TRN2 NeuronCores.

Sharding: batch x head-group data/tensor parallel. Core c handles batch c//4
and heads [4*(c%4), 4*(c%4)+4): it computes Q/K/V projections for its 256
feature columns, full attention for its 4 heads, and a partial output
projection against its 256 rows of W_o. The host sums the 4 partials per
batch (the "all-reduce after W_o" step of the sharding hint, done at
unshard time) and concatenates the two batches.

Numerics: the whole pre-softmax path runs in float32r (~2^-12 per-element
input rounding, fp32 accumulate). The resulting score error is ~0.3 in
scaled-score units; a noise study against the reference shows that level
of score noise costs ~8e-3 output Frobenius error (gate is 2e-2). The row
max m comes from a q-major f32r score pass reduced on DVE with fused
tensor_tensor_reduce pairs; it only needs to land within ~80 raw units of
the true max (any common shift cancels in softmax normalization). The
k-major score matmul subtracts m via an augmented contraction row
(kT row 64 = 1, qT row 64 = -m), so exp() fuses the PSUM->SBUF copy on
ScalarE with scale=1/sqrt(Dh). The softmax denominator comes free from an
appended ones-column on V; normalization is applied after the P@V matmul.
P is fp16 (post-softmax weights), V/att/W_o are f32r.
"""

from contextlib import ExitStack

import numpy as np

import concourse.bacc as bacc
import concourse.mybir as mybir
import concourse.tile as tile
from concourse import bass_utils
from concourse.masks import make_identity

AF = mybir.ActivationFunctionType
ALU = mybir.AluOpType
F32 = mybir.dt.float32
F16 = mybir.dt.float16
F32R = mybir.dt.float32r

B, S, E, H, Dh = 2, 2048, 1024, 16, 64
NCORES = 8
GROUPS = 4            # head groups (cores per batch)
HPC = H // GROUPS     # heads per core = 4
FG = HPC * Dh         # feature columns per core = 256
P = 128
SCALE = 1.0 / (Dh ** 0.5)

EO = E // P           # 8 contraction chunks
ST = S // P           # 16 sequence tiles of 128
QC = 512              # q-chunk width for the k-major score/PV pass
NQC = S // QC         # 4
NEG_INF = -3.0e38


def _emit(tc, debug=False):
    nc = tc.nc
    xt = nc.dram_tensor("xt", [E, S], F32R, kind="ExternalInput").ap()
    wq = nc.dram_tensor("wq", [E, FG], F32R, kind="ExternalInput").ap()
    wk = nc.dram_tensor("wk", [E, FG], F32R, kind="ExternalInput").ap()
    wv = nc.dram_tensor("wv", [E, FG], F32R, kind="ExternalInput").ap()
    wo = nc.dram_tensor("wo", [FG, E], F32R, kind="ExternalInput").ap()
    out = nc.dram_tensor("out", [S, E], F32, kind="ExternalOutput").ap()

    ctx = ExitStack()
    const = ctx.enter_context(tc.tile_pool(name="const", bufs=1))
    persist = ctx.enter_context(tc.tile_pool(name="persist", bufs=1))
    stage = ctx.enter_context(tc.tile_pool(name="stage", bufs=3))
    xqp = ctx.enter_context(tc.tile_pool(name="xqp", bufs=2))
    stgp = ctx.enter_context(tc.tile_pool(name="stgp", bufs=3))
    ptp = ctx.enter_context(tc.tile_pool(name="ptp", bufs=4))
    outp = ctx.enter_context(tc.tile_pool(name="outp", bufs=4))
    ps_stat = ctx.enter_context(tc.tile_pool(name="ps_stat", bufs=2, space="PSUM"))
    ps_sc = ctx.enter_context(tc.tile_pool(name="ps_sc", bufs=2, space="PSUM"))
    ps_pv = ctx.enter_context(tc.tile_pool(name="ps_pv", bufs=2, space="PSUM"))
    ps_wo = ctx.enter_context(tc.tile_pool(name="ps_wo", bufs=2, space="PSUM"))

    ident = const.tile([P, P], F32)
    make_identity(nc, ident[:])
    ones_f32 = const.tile([P, Dh], F32)
    nc.gpsimd.memset(ones_f32[:], 1.0)
    ones_mat = const.tile([P, Dh], F32R)
    nc.vector.tensor_copy(ones_mat[:], ones_f32[:])

    # persistent SBUF tensors
    wqs = persist.tile([P, EO, FG], F32R)
    wks = persist.tile([P, EO, FG], F32R)
    wvs = persist.tile([P, EO, FG], F32R)
    wos = persist.tile([P, FG // P, E], F32R)
    # per-head Q^T/K^T (partitions 0-63 data; row 64 = -m on Q, ones on K)
    qT = persist.tile([P, HPC, S], F32R)
    kT = persist.tile([P, HPC, S], F32R)
    # V with appended ones column, k-major (partitions = sequence position)
    vau = persist.tile([P, ST, HPC, Dh + 1], F16)
    # normalized attention output, feature-major: feature fc*128+p, q free
    att = persist.tile([P, FG // P, S], F32R)

    xt_re = xt.rearrange("(eo p) s -> p eo s", p=P)
    _wks_re = wk.rearrange("(eo p) m -> p eo m", p=P)
    nc.sync.dma_start(wks[:, 0:2, :], _wks_re[:, 0:2, :])
    nc.sync.dma_start(wks[:, 2:, :], _wks_re[:, 2:, :])

    # K-aug row = 1 so the q-side aug row (-m, raw units) lands in every score
    nc.gpsimd.memset(kT[Dh : Dh + 1, :, :].bitcast(F32), 1.0)
    nc.gpsimd.memset(vau[:, :, :, Dh : Dh + 1], 1.0)

    maxs = [
        stage.tile([P, ST], F32, tag=f"maxs{h}", name=f"maxs{h}") for h in range(HPC)
    ]

    # q-major f32r score pass for the row max, reduced per 512-block on DVE
    def stats_steps(h, qt_order=None):
        mx = maxs[h]
        for qt in qt_order or range(ST):
            hm = stage.tile([P, 4], F32, tag="hm")
            for kc in range(4):
                ps = ps_stat.tile([P, 512], F32, tag="stat", name="ps_stat")
                yield lambda ps=ps, qt=qt, h=h, kc=kc: nc.tensor.matmul(
                    ps[:],
                    lhsT=qT[0:Dh, h, qt * P : (qt + 1) * P],
                    rhs=kT[0:Dh, h, kc * 512 : (kc + 1) * 512],
                    start=True,
                    stop=True,
                )
                nc.vector.reduce_max(
                    hm[:, kc : kc + 1], ps[:], axis=mybir.AxisListType.X
                )
            nc.vector.tensor_reduce(
                mx[:, qt : qt + 1], hm[:, 0:4], axis=mybir.AxisListType.X,
                op=ALU.max,
            )

    def drain(it, n=1 << 30):
        k = 0
        if it is not None:
            for step in it:
                step()
                k += 1
                if k >= n:
                    break

    Q_CHUNK_ORDER = [2, 3, 1, 0]
    qt0_order = [qc4 * 4 + j for qc4 in Q_CHUNK_ORDER for j in range(4)]
    stats_its = [stats_steps(0, qt0_order)] + [stats_steps(h) for h in range(1, HPC)]

    # transpose the per-head row-max vector into the qT aug row (negated)
    def aug_prep(h):
        psm = ps_wo.tile([P, QC], F32, tag="wo", name="psm")
        nc.tensor.transpose(psm[0:ST, 0:P], maxs[h][:, :], ident[:])
        mst = stage.tile([ST, P], F32R, tag="mst")
        nc.scalar.mul(mst[:], psm[0:ST, 0:P], -1.0)
        nc.sync.dma_start(qT[Dh : Dh + 1, h, :], mst[:, :])

    # ---- phase 1: K + V projections share each x chunk
    xq_tiles = {}
    for qc4 in range(NQC):
        xq = xqp.tile([P, EO, QC], F32R, tag="xq")
        xq_tiles[qc4] = xq
        qs = slice(qc4 * QC, (qc4 + 1) * QC)
        if qc4 == 0:
            for e2 in range(0, EO, 2):
                nc.sync.dma_start(xq[:, e2 : e2 + 2, :], xt_re[:, e2 : e2 + 2, qs])
            # late weights ride the HWDGE queue behind the first x chunk;
            # they must be EMITTED before the first V/Q matmuls that read them
            nc.gpsimd.dma_start(wvs[:], wv.rearrange("(eo p) m -> p eo m", p=P))
            nc.gpsimd.dma_start(wqs[:], wq.rearrange("(eo p) m -> p eo m", p=P))
            nc.gpsimd.dma_start(wos[:], wo.rearrange("(fo p) e -> p fo e", p=P))
        else:
            nc.sync.dma_start(xq[:], xt_re[:, :, qs])
        for mc in range(FG // P):
            ps = ps_sc.tile([P, QC], F32, tag="sc", name="ps_kproj")
            for eo in range(EO):
                nc.tensor.matmul(
                    ps,
                    lhsT=wks[:, eo, mc * P : (mc + 1) * P],
                    rhs=xq[:, eo, :],
                    start=(eo == 0),
                    stop=(eo == EO - 1),
                )
            stg = stgp.tile([P, QC], F32R, tag="stg")
            nc.vector.tensor_copy(stg[:], ps)
            for hh in range(2):
                h = mc * 2 + hh
                nc.sync.dma_start(kT[0:Dh, h, qs], stg[hh * Dh : (hh + 1) * Dh, :])
        for st4 in range(4):
            st = qc4 * 4 + st4
            ps = ps_pv.tile([P, QC], F32, tag="pv", name="ps_v")[:, :FG]
            for eo in range(EO):
                nc.tensor.matmul(
                    ps,
                    lhsT=xq[:, eo, st4 * P : (st4 + 1) * P],
                    rhs=wvs[:, eo, :],
                    start=(eo == 0),
                    stop=(eo == EO - 1),
                )
            nc.scalar.copy(
                vau[:, st, :, 0:Dh],
                ps.rearrange("p (h d) -> p h d", h=HPC),
            )

    # ---- phase 2: Q projection; chunks 2,3 still resident in the xq ring.
    # head-0 stats drain here (16 per chunk, matching qt availability).
    for ci, qc4 in enumerate(Q_CHUNK_ORDER):
        if qc4 in (2, 3):
            xq = xq_tiles[qc4]
        else:
            xq = xqp.tile([P, EO, QC], F32R, tag="xq")
            nc.sync.dma_start(xq[:], xt_re[:, :, qc4 * QC : (qc4 + 1) * QC])
        qs = slice(qc4 * QC, (qc4 + 1) * QC)
        for mc in range(FG // P):
            ps = ps_sc.tile([P, QC], F32, tag="sc", name="ps_qproj")
            for eo in range(EO):
                nc.tensor.matmul(
                    ps,
                    lhsT=wqs[:, eo, mc * P : (mc + 1) * P],
                    rhs=xq[:, eo, :],
                    start=(eo == 0),
                    stop=(eo == EO - 1),
                )
            stg = stgp.tile([P, QC], F32R, tag="stg")
            nc.vector.tensor_copy(stg[:], ps)
            for hh in range(2):
                h = mc * 2 + hh
                nc.sync.dma_start(qT[0:Dh, h, qs], stg[hh * Dh : (hh + 1) * Dh, :])
            drain(stats_its[0], 8)
    drain(stats_its[0])
    aug_prep(0)

    # ---- per head: k-major scores -> exp -> PV, software-pipelined so PE
    # never sits behind the Act exp; heads 2/3 stats and the W_o projection
    # interleave into the PE slack of the Act-bound chain.
    def wo_steps(qt_lo, qt_hi):
        for qt in range(qt_lo, qt_hi):
            for ec in range(E // QC):
                ps = ps_wo.tile([P, QC], F32, tag="wo", name="ps_wo")
                for fc in range(FG // P):
                    yield lambda ps=ps, qt=qt, ec=ec, fc=fc: nc.tensor.matmul(
                        ps,
                        lhsT=att[:, fc, qt * P : (qt + 1) * P],
                        rhs=wos[:, fc, ec * QC : (ec + 1) * QC],
                        start=(fc == 0),
                        stop=(fc == FG // P - 1),
                        skip_group_check=True,
                    )
                ob = outp.tile([P, QC], F32, tag="ob")
                nc.vector.tensor_copy(ob[:], ps)
                nc.sync.dma_start(
                    out[qt * P : (qt + 1) * P, ec * QC : (ec + 1) * QC], ob[:]
                )

    LAG = 2
    head_order = [0, 1, 2, 3]
    # Fill schedule: head h's section drains head h+1's stats, one step per
    # kt slot (64 steps / 64 slots); the last head drains W_o instead.
    fills = {0: stats_its[1], 1: stats_its[2], 2: stats_its[3]}
    wo_pending = None
    for hi_idx, h in enumerate(head_order):
        for qc in range(NQC):
            qs = slice(qc * QC, (qc + 1) * QC)
            pv = ps_pv.tile([P, QC], F32, tag="pv")
            pts = {}

            def pv_mm(kt, pv=pv, h=h):
                nc.tensor.matmul(
                    pv[0 : Dh + 1, :],
                    lhsT=vau[:, kt, h, :],
                    rhs=pts.pop(kt)[:],
                    start=(kt == 0),
                    stop=(kt == ST - 1),
                    skip_group_check=True,
                )

            for kt in range(ST):
                ks = slice(kt * P, (kt + 1) * P)
                if hi_idx == HPC - 1:
                    drain(wo_pending, 1)
                else:
                    drain(fills[h], 1)
                ps = ps_sc.tile([P, QC], F32, tag="sc")
                nc.tensor.matmul(
                    ps, lhsT=kT[0 : Dh + 1, h, ks], rhs=qT[0 : Dh + 1, h, qs],
                    start=True, stop=True,
                )
                pt = ptp.tile([P, QC], F16, tag="pt")
                pts[kt] = pt
                nc.scalar.activation(pt[:], ps[:], AF.Exp, scale=SCALE)
                if kt >= LAG:
                    pv_mm(kt - LAG)
            for kt in range(ST - LAG, ST):
                pv_mm(kt)
            li = stage.tile([P, QC], F32R, tag="li")
            with nc.allow_low_precision(reason="1/l in f32r (~2^-12) is ample"):
                nc.vector.reciprocal(li[Dh : Dh + 1, :], pv[Dh : Dh + 1, :])
            pb = ps_wo.tile([P, QC], F32, tag="wo", name="pb")
            nc.tensor.matmul(
                pb[0:Dh, :], lhsT=ones_mat[Dh : Dh + 1, :], rhs=li[Dh : Dh + 1, :],
                start=True, stop=True,
            )
            bc = stage.tile([P, QC], F32, tag="bc")
            nc.scalar.copy(bc[0:Dh, :], pb[0:Dh, :])
            if h % 2 == 0:
                nc.vector.tensor_tensor(
                    att[0:Dh, h // 2, qs], pv[0:Dh, :], bc[0:Dh, :], ALU.mult
                )
            else:
                stg = stage.tile([P, QC], F32R, tag="att_stg")
                nc.vector.tensor_tensor(stg[0:Dh, :], pv[0:Dh, :], bc[0:Dh, :], ALU.mult)
                nc.sync.dma_start(att[Dh : 2 * Dh, h // 2, qs], stg[0:Dh, :])
            if hi_idx == HPC - 1:
                drain(wo_pending)
                wo_pending = wo_steps(4 * qc, 4 * qc + 4)
        if hi_idx == HPC - 1:
            drain(wo_pending)
        if hi_idx + 1 < HPC:
            nxt = head_order[hi_idx + 1]
            drain(stats_its[nxt])
            aug_prep(nxt)

    if debug:
        vau_d = nc.dram_tensor("vau_d", [P, ST, HPC, Dh + 1], F16, kind="ExternalOutput").ap()
        nc.sync.dma_start(vau_d, vau[:])
        att_d = nc.dram_tensor("att_d", [P, FG // P, S], F32, kind="ExternalOutput").ap()
        qT_d = nc.dram_tensor("qT_d", [P, HPC, S], F32, kind="ExternalOutput").ap()
        kT_d = nc.dram_tensor("kT_d", [P, HPC, S], F32, kind="ExternalOutput").ap()
        nc.sync.dma_start(att_d, att[:].bitcast(F32))
        nc.sync.dma_start(qT_d, qT[:].bitcast(F32))
        nc.sync.dma_start(kT_d, kT[:].bitcast(F32))
    ctx.close()


_NC = None


def _build(debug=False):
    global _NC
    if debug:
        nc = bacc.Bacc(
            "TRN2", target_bir_lowering=False, debug=False, num_devices=NCORES
        )
        with tile.TileContext(nc) as tc:
            _emit(tc, debug=True)
        nc.compile()
        return nc
    if _NC is None:
        nc = bacc.Bacc(
            "TRN2", target_bir_lowering=False, debug=False, num_devices=NCORES
        )
        with tile.TileContext(nc) as tc:
            _emit(tc)
        nc.compile()
        _NC = nc
    return _NC


def _prep_inputs(x, W_q, W_k, W_v, W_o):
    x = np.asarray(x, dtype=np.float32)
    W_q = np.asarray(W_q, dtype=np.float32)
    W_k = np.asarray(W_k, dtype=np.float32)
    W_v = np.asarray(W_v, dtype=np.float32)
    W_o = np.asarray(W_o, dtype=np.float32)

    xts = [np.ascontiguousarray(x[b].T) for b in range(B)]
    in_maps = []
    for c in range(NCORES):
        b, g = divmod(c, GROUPS)
        fg = slice(g * FG, (g + 1) * FG)
        in_maps.append(
            {
                "xt": xts[b],
                "wq": np.ascontiguousarray(W_q[:, fg]),
                "wk": np.ascontiguousarray(W_k[:, fg]),
                "wv": np.ascontiguousarray(W_v[:, fg]),
                "wo": np.ascontiguousarray(W_o[fg, :]),
            }
        )
    return in_maps


def run(inputs, **spmd_kwargs):
    nc = _build()
    in_maps = _prep_inputs(
        inputs["x"], inputs["W_q"], inputs["W_k"], inputs["W_v"], inputs["W_o"]
    )
    res = bass_utils.run_bass_kernel_spmd(
        nc, in_maps, core_ids=list(range(NCORES)), **spmd_kwargs
    )
    out = np.zeros((B, S, E), dtype=np.float32)
    for c in range(NCORES):
        out[c // GROUPS] += res.results[c]["out"]
    return out, res


def kernel(**inputs):
    out, _ = run(inputs)
    return out
